# revision 24
# baseline (speedup 1.0000x reference)
"""Trainium2 Bass kernel for nn_CausalWanModel (frame-block-causal attention).

Head-parallel (tensor parallel) across 8 cores, 12 heads:
  core c owns FULL head F[c] (all queries) plus one query-half of SHARED
  head S[c] (half hc = c%2 of every frame: tokens [780f+390hc, +390)).
  Every core computes q/k/v projections for its 2 heads locally over all
  tokens (k/v of shared heads computed redundantly on 2 cores), so there
  is no k/v all-gather.  Block-causal attention skips masked frames
  entirely (query frame f attends key prefix [0, 780(f+1))).
  RMSNorm statistics need all 1536 features -> tiny [2,T] f32 AllReduce
  of per-core partial sum-of-squares, overlapped with the v projection.
  After attention, an AllToAll routes exactly the attention slices each
  core needs for its 390-token output chunk; o-projection is computed
  redundantly per output chunk with full Wo (zero-padded selector for
  shared-head pieces), followed by a direct PSUM->DRAM store.

Shapes hardcoded: B=1, T=3120, D=1536, H=12, hd=128, frame_seqlen=780.
"""

import math

import numpy as np
import ml_dtypes

import concourse.bacc as bacc
import concourse.mybir as mybir
import concourse.tile as tile
from concourse.bass_utils import run_bass_kernel_spmd

F32 = mybir.dt.float32
F16 = mybir.dt.float16
BF16 = mybir.dt.bfloat16

NC = 8
T = 3120
D = 1536
KC = 12           # contraction chunks of 128 over D
H = 12
HD = 128
L = 780           # frame_seqlen
NF = 4            # frames
QC = 390          # query chunk (half frame)
NCH = 8           # natural token chunks of 390
NKT = 25          # key tiles of 128 covering padded T
TP = NKT * 128    # 3200 padded token count for k/v tiles
EPS = 1e-6
SCALE = 1.0 / math.sqrt(HD)

FULLH = [0, 2, 3, 5, 6, 8, 9, 11]
SHARH = [1, 1, 4, 4, 7, 7, 10, 10]
NTILES = [7, 13, 19, 25]        # key tiles for prefix 780*(f+1)
TAILV = [12, 24, 36, 48]        # valid rows in the tail key tile


def build_kernel():
    nc = bacc.Bacc("TRN2", target_bir_lowering=False, debug=False,
                   num_devices=NC)

    # ---------------- DRAM I/O ----------------
    xT = nc.dram_tensor("xT", [128, KC, T], BF16, kind="ExternalInput")
    xh2 = nc.dram_tensor("xh2", [128, KC, NF * QC], BF16, kind="ExternalInput")
    wq = nc.dram_tensor("wq", [128, 2, KC, 128], BF16, kind="ExternalInput")
    wk = nc.dram_tensor("wk", [128, 2, KC, 128], BF16, kind="ExternalInput")
    wv = nc.dram_tensor("wv", [128, 2, KC, 128], BF16, kind="ExternalInput")
    wosel = nc.dram_tensor("wosel", [KC, 128, 16, 128], BF16,
                           kind="ExternalInput")
    biasqkv = nc.dram_tensor("biasqkv", [128, 6], F32, kind="ExternalInput")
    mqw = nc.dram_tensor("mqw", [128, 2, 128], BF16, kind="ExternalInput")
    cosN = nc.dram_tensor("cosN", [128, T], F16, kind="ExternalInput")
    sinN = nc.dram_tensor("sinN", [128, T], F16, kind="ExternalInput")
    cosS = nc.dram_tensor("cosS", [128, NF * QC], F16, kind="ExternalInput")
    sinS = nc.dram_tensor("sinS", [128, NF * QC], F16, kind="ExternalInput")
    maskb = nc.dram_tensor("maskb", [128, NF], F32, kind="ExternalInput")
    maskq = nc.dram_tensor("maskq", [128, NCH], F32, kind="ExternalInput")
    outT = nc.dram_tensor("outT", [D, QC], F32, kind="ExternalOutput")

    # collective buffers
    dummy_in = nc.dram_tensor("dummy_in", [1, 64], BF16)
    dummy_out = nc.dram_tensor("dummy_out", [NC, 64], BF16,
                               addr_space="Shared")
    ssq_k = nc.dram_tensor("ssq_k", [1, T], BF16)
    ssq_ko = nc.dram_tensor("ssq_ko", [NC, T], BF16, addr_space="Shared")
    ssq_qa = nc.dram_tensor("ssq_qa", [1, 4 * QC], BF16)
    ssq_qao = nc.dram_tensor("ssq_qao", [NC, 4 * QC], BF16,
                             addr_space="Shared")
    ssq_qb = nc.dram_tensor("ssq_qb", [1, 4 * QC], BF16)
    ssq_qbo = nc.dram_tensor("ssq_qbo", [NC, 4 * QC], BF16,
                             addr_space="Shared")
    a2a_in = nc.dram_tensor("a2a_in", [NC, 2, 128, QC], BF16)
    a2a_out = nc.dram_tensor("a2a_out", [NC, 2, 128, QC], BF16)

    FW = 7 * 128  # per-frame padded key width (896)
    RG = [list(range(NC))]

    with tile.TileContext(nc) as tc:
        with tc.tile_pool(name="const", bufs=1) as cpool:
            kT_t = [[cpool.tile([128, FW], BF16, tag=f"kT_{s}_{f}",
                                name=f"kT_{s}_{f}")
                     for f in range(NF)] for s in range(2)]
            v_t = [[cpool.tile([128, 7, 128], BF16, tag=f"v_{s}_{f}",
                               name=f"v_{s}_{f}")
                    for f in range(NF)] for s in range(2)]
            qf_t = [cpool.tile([128, L], BF16, tag=f"qf_{f}", name=f"qf_{f}")
                    for f in range(NF)]
            qs_t = [cpool.tile([128, QC], BF16, tag=f"qs_{f}", name=f"qs_{f}")
                    for f in range(NF)]
            u_q = cpool.tile([128, KC, QC], BF16, tag="u_q")  # 8 nat + 4 slot
            u_k = cpool.tile([128, 2, NCH, QC], BF16, tag="u_k")
            ones128 = cpool.tile([128, 128], BF16, tag="ones128")
            half128 = cpool.tile([128, 128], BF16, tag="half128")
            ones8 = cpool.tile([8, 128], BF16, tag="ones8")
            mq_sb = cpool.tile([128, 2, 128], BF16, tag="mq_sb")
            bias_sb = cpool.tile([128, 6], F32, tag="bias_sb")
            maskb_sb = cpool.tile([128, NF], F32, tag="maskb_sb")
            maskq_sb = cpool.tile([128, NCH], F32, tag="maskq_sb")
            eps_sb = cpool.tile([128, 1], F32, tag="eps_sb")
            cosN_s = cpool.tile([128, T], F16, tag="cosN_s")
            sinN_s = cpool.tile([128, T], F16, tag="sinN_s")
            cosS_s = cpool.tile([128, NF * QC], F16, tag="cosS_s")
            sinS_s = cpool.tile([128, NF * QC], F16, tag="sinS_s")

            nc.gpsimd.memset(ones128[:, :], 1.0)
            nc.gpsimd.memset(half128[:, :], 0.5)
            nc.gpsimd.memset(ones8[:, :], 1.0)
            nc.gpsimd.memset(eps_sb[:, :], EPS)
            nc.sync.dma_start(out=bias_sb[:, :], in_=biasqkv[:, :])
            nc.sync.dma_start(out=maskb_sb[:, :], in_=maskb[:, :])
            nc.sync.dma_start(out=mq_sb[:, :, :], in_=mqw[:, :, :])
            nc.sync.dma_start(out=maskq_sb[:, :], in_=maskq[:, :])
            # rendezvous: absorb cross-core startup skew behind projections
            nc.gpsimd.collective_compute(
                "AllGather", mybir.AluOpType.bypass,
                ins=[dummy_in.ap().opt()], outs=[dummy_out.ap().opt()],
                replica_groups=RG)
            for s in range(2):
                for f in range(NF):
                    nc.vector.memset(kT_t[s][f][:, L:FW], 0.0)

            # ============ Phase P: v, k, q projections ============
            with tc.tile_pool(name="xpool", bufs=1) as xpool, \
                 tc.tile_pool(name="wpool", bufs=1) as wpool, \
                 tc.tile_pool(name="x2p", bufs=2) as x2p, \
                 tc.tile_pool(name="vstg", bufs=2) as vstg, \
                 tc.tile_pool(name="sqtile", bufs=3) as sqtile, \
                 tc.tile_pool(name="sqhs", bufs=1) as sqhsp, \
                 tc.tile_pool(name="sqst", bufs=3) as sqstp, \
                 tc.tile_pool(name="rtmp", bufs=2) as rtmp, \
                 tc.tile_pool(name="pps", bufs=4, space="PSUM") as pps, \
                 tc.tile_pool(name="sqps", bufs=2, space="PSUM") as sqps:

                wq_s = wpool.tile([128, 2, KC, 128], BF16, tag="wq_s")
                wk_s = wpool.tile([128, 2, KC, 128], BF16, tag="wk_s")
                wv_s = wpool.tile([128, 2, KC, 128], BF16, tag="wv_s")
                nc.sync.dma_start(out=wv_s[:, :, :, :], in_=wv[:, :, :, :])
                nc.scalar.dma_start(out=wk_s[:, :, :, :], in_=wk[:, :, :, :])
                nc.scalar.dma_start(out=wq_s[:, :, :, :], in_=wq[:, :, :, :])
                xT_s = xpool.tile([128, KC, T], BF16, tag="xT_s")
                for ch in range(NCH):
                    nc.sync.dma_start(
                        out=xT_s[:, :, ch * QC:(ch + 1) * QC],
                        in_=xT[:, :, ch * QC:(ch + 1) * QC])
                nc.sync.dma_start(out=cosN_s[:, :], in_=cosN[:, :])
                nc.sync.dma_start(out=sinN_s[:, :], in_=sinN[:, :])
                nc.sync.dma_start(out=cosS_s[:, :], in_=cosS[:, :])
                nc.sync.dma_start(out=sinS_s[:, :], in_=sinS[:, :])
                x2_t = []
                for f in range(NF):
                    x2 = x2p.tile([128, KC, QC], BF16, tag="x2", name="x2")
                    nc.sync.dma_start(out=x2[:, :, :],
                                      in_=xh2[:, :, f * QC:(f + 1) * QC])
                    x2_t.append(x2)

                def proj_chunk(w_s, s, rhs_ap):
                    ps = pps.tile([128, QC], F32, tag="proj_ps",
                                  name="proj_ps")
                    for dd in range(KC):
                        nc.tensor.matmul(ps[:, :], w_s[:, s, dd, :],
                                         rhs_ap[:, dd, :],
                                         start=(dd == 0), stop=(dd == KC - 1))
                    return ps

                # ---- v projections first (clears ACT queue early) ----
                for s in range(2):
                    for f in range(NF):
                        vt = vstg.tile([128, FW], BF16, tag="vstg",
                                       name="vstg")
                        for half in range(2):
                            ch = 2 * f + half
                            ps = proj_chunk(
                                wv_s, s, xT_s[:, :, ch * QC:(ch + 1) * QC])
                            nc.scalar.activation(
                                vt[:, half * QC:(half + 1) * QC], ps[:, :],
                                mybir.ActivationFunctionType.Identity,
                                bias=bias_sb[:, 4 + s:5 + s])
                        nc.vector.memset(vt[:, L:FW], 0.0)
                        nc.sync.dma_start_transpose(v_t[s][f][:, :, :],
                                                    vt[:, :])

                # ---- k projections + ssq_k ----
                for ch in range(NCH):
                    rhs = xT_s[:, :, ch * QC:(ch + 1) * QC]
                    sqp = sqps.tile([128, QC], F32, tag="ssqk", name="ssqk")
                    for s in range(2):
                        ps = proj_chunk(wk_s, s, rhs)
                        ue = u_k[:, s, ch, :]
                        nc.scalar.activation(
                            ue, ps[:, :],
                            mybir.ActivationFunctionType.Identity,
                            bias=bias_sb[:, 2 + s:3 + s])
                        sq = sqtile.tile([128, QC], BF16, tag="sq", name="sq")
                        nc.vector.tensor_tensor(sq[:, :], ue, ue,
                                                mybir.AluOpType.mult)
                        nc.tensor.matmul(
                            sqp[:, :],
                            ones128[:, :] if s == 0 else half128[:, :],
                            sq[:, :], start=(s == 0), stop=(s == 1))
                    st = sqstp.tile([1, QC], BF16, tag="sqst", name="sqst")
                    nc.scalar.copy(st[:, :], sqp[0:1, :])
                    nc.sync.dma_start(
                        out=ssq_k.ap()[0:1, ch * QC:(ch + 1) * QC],
                        in_=st[:, :])
                nc.gpsimd.collective_compute(
                    "AllGather", mybir.AluOpType.bypass,
                    ins=[ssq_k.ap().opt()], outs=[ssq_ko.ap().opt()],
                    replica_groups=RG)

                # ---- rope for k (no norm scale; overlaps q-proj) ----
                def rope_raw(u_ap, w, cos_ap, sin_ap, dst_ap):
                    t0s = rtmp.tile([128, w], BF16, tag="t0s", name="t0s")
                    nc.scalar.dma_start(out=t0s[0:64, :], in_=u_ap[64:128, :])
                    nc.scalar.dma_start(out=t0s[64:128, :], in_=u_ap[0:64, :])
                    t2 = rtmp.tile([128, w], BF16, tag="t2", name="t2")
                    nc.vector.tensor_tensor(dst_ap, u_ap, cos_ap,
                                            mybir.AluOpType.mult)
                    nc.vector.tensor_tensor(t2[:, :], t0s[:, :], sin_ap,
                                            mybir.AluOpType.mult)
                    nc.vector.tensor_tensor(dst_ap, dst_ap, t2[:, :],
                                            mybir.AluOpType.add)

                # ---- q shared-head (slot chunks) ----
                sq_hs = []
                for f in range(NF):
                    ps = proj_chunk(wq_s, 1, x2_t[f][:, :, :])
                    ue = u_q[:, NCH + f, :]
                    nc.scalar.activation(ue, ps[:, :],
                                         mybir.ActivationFunctionType.Identity,
                                         bias=bias_sb[:, 1:2])
                    sqt = sqhsp.tile([128, QC], BF16, tag=f"sqhs{f}",
                                     name=f"sqhs{f}")
                    nc.vector.tensor_tensor(sqt[:, :], ue, ue,
                                            mybir.AluOpType.mult)
                    sq_hs.append(sqt)

                for s in range(2):
                    for f in range(NF):
                        o = f * L
                        rope_raw(u_k[:, s, 2 * f:2 * f + 2, :].rearrange(
                                     "p a b -> p (a b)"), L,
                                 cosN_s[:, o:o + L], sinN_s[:, o:o + L],
                                 kT_t[s][f][:, 0:L])

                # ---- q full head + combined ssq, in 2 AG halves ----
                for ch in range(NCH):
                    rhs = xT_s[:, :, ch * QC:(ch + 1) * QC]
                    ps = proj_chunk(wq_s, 0, rhs)
                    ue = u_q[:, ch, :]
                    nc.scalar.activation(ue, ps[:, :],
                                         mybir.ActivationFunctionType.Identity,
                                         bias=bias_sb[:, 0:1])
                    # square on ACT (keeps DVE free for rope)
                    sq = sqtile.tile([128, QC], BF16, tag="sq", name="sq")
                    nc.scalar.activation(sq[:, :], ue,
                                         mybir.ActivationFunctionType.Square)
                    sqp = sqps.tile([128, QC], F32, tag="ssqq", name="ssqq")
                    nc.tensor.matmul(sqp[:, :], ones128[:, :], sq[:, :],
                                     start=True, stop=False)
                    nc.tensor.matmul(sqp[:, :], mq_sb[:, ch % 2, :],
                                     sq_hs[ch // 2][:, :],
                                     start=False, stop=True)
                    st = sqstp.tile([1, QC], BF16, tag="sqst", name="sqst")
                    nc.scalar.copy(st[:, :], sqp[0:1, :])
                    buf = ssq_qa if ch < 4 else ssq_qb
                    nc.sync.dma_start(
                        out=buf.ap()[0:1, (ch % 4) * QC:(ch % 4 + 1) * QC],
                        in_=st[:, :])
                    if ch == 3:
                        nc.gpsimd.collective_compute(
                            "AllGather", mybir.AluOpType.bypass,
                            ins=[ssq_qa.ap().opt()],
                            outs=[ssq_qao.ap().opt()], replica_groups=RG)
                nc.gpsimd.collective_compute(
                    "AllGather", mybir.AluOpType.bypass,
                    ins=[ssq_qb.ap().opt()], outs=[ssq_qbo.ap().opt()],
                    replica_groups=RG)

                # ---- rope for q ----
                for f in range(NF):
                    o = f * L
                    rope_raw(u_q[:, 2 * f:2 * f + 2, :].rearrange(
                                 "p a b -> p (a b)"), L,
                             cosN_s[:, o:o + L], sinN_s[:, o:o + L],
                             qf_t[f][:, :])
                    rope_raw(u_q[:, NCH + f, :], QC,
                             cosS_s[:, f * QC:(f + 1) * QC],
                             sinS_s[:, f * QC:(f + 1) * QC],
                             qs_t[f][:, :])

            # ====== phase 2: inv chains + attention ======
            with tc.tile_pool(name="p2", bufs=1) as p2:
                inv_q = p2.tile([128, T], F16, tag="inv_q")
                inv_k = p2.tile([128, T], F16, tag="inv_k")
                invsl = p2.tile([128, NF * QC], F16, tag="invsl")
                wosel_sb = p2.tile([128, KC, 16, 128], BF16, tag="wosel_sb")

                with tc.tile_pool(name="agp", bufs=1) as agp, \
                     tc.tile_pool(name="agt", bufs=3) as agt, \
                     tc.tile_pool(name="warmps", bufs=1, space="PSUM") as wps, \
                     tc.tile_pool(name="agps", bufs=3, space="PSUM") as agps:
                    # warm MMs bridge the proj->attention HAM gap
                    warm = wps.tile([128, 390], F32, tag="warm")
                    for _ in range(40):
                        nc.tensor.matmul(warm[:, :], ones128[:, :],
                                         cosN_s[:, 0:390], start=True,
                                         stop=True)

                    def inv_chain(agbuf_ap, nch, dst):
                        for j in range(nch):
                            sp = agps.tile([128, QC], F32, tag="sp",
                                           name="sp")
                            nc.tensor.matmul(sp[:, :], ones8[:, :],
                                             agbuf_ap[:, j * QC:(j + 1) * QC],
                                             start=True, stop=True)
                            rmsj = agt.tile([128, QC], F32, tag="rmsj",
                                            name="rmsj")
                            nc.scalar.activation(
                                rmsj[:, :], sp[:, :],
                                mybir.ActivationFunctionType.Sqrt,
                                bias=eps_sb[:, :], scale=1.0 / D)
                            invj = agt.tile([128, QC], F32, tag="invj",
                                            name="invj")
                            nc.vector.reciprocal(invj[:, :], rmsj[:, :])
                            nc.vector.tensor_copy(
                                dst[:, j * QC:(j + 1) * QC], invj[:, :])

                    agbk = agp.tile([8, T], BF16, tag="agbk")
                    nc.sync.dma_start(out=agbk[:, :], in_=ssq_ko.ap()[:, :])
                    inv_chain(agbk, NCH, inv_k)
                    agba = agp.tile([8, 4 * QC], BF16, tag="agba")
                    nc.sync.dma_start(out=agba[:, :], in_=ssq_qao.ap()[:, :])
                    inv_chain(agba, 4, inv_q[:, 0:4 * QC])
                    agbb = agp.tile([8, 4 * QC], BF16, tag="agbb")
                    nc.sync.dma_start(out=agbb[:, :], in_=ssq_qbo.ap()[:, :])
                    inv_chain(agbb, 4, inv_q[:, 4 * QC:T])
                    for f in range(NF):
                        t1 = agt.tile([128, QC], F16, tag="isl1", name="isl1")
                        nc.vector.tensor_scalar_mul(
                            t1[:, :],
                            inv_q[:, (2 * f) * QC:(2 * f + 1) * QC],
                            maskq_sb[:, 2 * f:2 * f + 1])
                        t2 = agt.tile([128, QC], F16, tag="isl2", name="isl2")
                        nc.vector.tensor_scalar_mul(
                            t2[:, :],
                            inv_q[:, (2 * f + 1) * QC:(2 * f + 2) * QC],
                            maskq_sb[:, 2 * f + 1:2 * f + 2])
                        nc.vector.tensor_tensor(
                            invsl[:, f * QC:(f + 1) * QC],
                            t1[:, :], t2[:, :], mybir.AluOpType.add)

                # ====== norm-scale + attention, pipelined per frame ======
                with tc.tile_pool(name="scps", bufs=2, space="PSUM") as scps, \
                     tc.tile_pool(name="denps", bufs=1, space="PSUM") as denps, \
                     tc.tile_pool(name="accps", bufs=1, space="PSUM") as accps, \
                     tc.tile_pool(name="prp", bufs=3) as prp, \
                     tc.tile_pool(name="astg", bufs=3) as astg:

                    def scale_ip(dst_ap, inv_ap):
                        nc.vector.tensor_tensor(dst_ap, dst_ap, inv_ap,
                                                mybir.AluOpType.mult)

                    for f in range(NF):
                        o = f * L
                        for s in range(2):
                            scale_ip(kT_t[s][f][:, 0:L],
                                     inv_k[:, o:o + L])
                        scale_ip(qf_t[f][:, :], inv_q[:, o:o + L])
                        scale_ip(qs_t[f][:, :], invsl[:, f * QC:(f + 1) * QC])

                        for s in range(3):
                            hs = 0 if s < 2 else 1
                            if s < 2:
                                qsrc = qf_t[f][:, s * QC:(s + 1) * QC]
                            else:
                                qsrc = qs_t[f][:, :]
                            den = denps.tile([128, QC], F32, tag="den",
                                             name="den")
                            acc = accps.tile([128, QC], F32, tag="acc",
                                             name="acc")
                            for fr in range(f + 1):
                                for (g0, gsz) in ((0, 3), (3, 3), (6, 1)):
                                    is_tail = (g0 == 6)
                                    sc = scps.tile([128, 3, 512], F32,
                                                   tag="sc", name="sc")
                                    for j in range(gsz):
                                        kt = g0 + j
                                        nc.tensor.matmul(
                                            sc[:, j, 0:QC],
                                            kT_t[hs][fr][:, kt * 128:
                                                         (kt + 1) * 128],
                                            qsrc, start=True, stop=True)
                                    pr = prp.tile([128, 3, QC], BF16,
                                                  tag="pr", name="pr")
                                    nc.scalar.activation(
                                        pr[:, 0:gsz, :], sc[:, 0:gsz, 0:QC],
                                        mybir.ActivationFunctionType.Exp,
                                        bias=(maskb_sb[:, 0:1] if is_tail
                                              else 0.0),
                                        scale=SCALE)
                                    for j in range(gsz):
                                        kt = g0 + j
                                        first = (fr == 0 and kt == 0)
                                        last = (fr == f and kt == 6)
                                        nc.tensor.matmul(
                                            den[:, :], ones128[:, :],
                                            pr[:, j, :],
                                            start=first, stop=last)
                                    for j in range(gsz):
                                        kt = g0 + j
                                        first = (fr == 0 and kt == 0)
                                        last = (fr == f and kt == 6)
                                        nc.tensor.matmul(
                                            acc[:, :], v_t[hs][fr][:, kt, :],
                                            pr[:, j, :],
                                            start=first, stop=last)
                            recb = astg.tile([128, QC], F32, tag="recb",
                                             name="recb")
                            nc.vector.reciprocal(recb[:, :], den[:, :])
                            att = astg.tile([128, QC], BF16, tag="att",
                                            name="att")
                            nc.vector.tensor_tensor(att[:, :], acc[:, :],
                                                    recb[:, :],
                                                    mybir.AluOpType.mult)
                            if s < 2:
                                nc.sync.dma_start(
                                    out=a2a_in.ap()[2 * f + s, 0, :, :],
                                    in_=att[:, :])
                            else:
                                nc.sync.dma_start(
                                    out=a2a_in.ap()[2 * f, 1, :, :],
                                    in_=att[:, :])
                                nc.sync.dma_start(
                                    out=a2a_in.ap()[2 * f + 1, 1, :, :],
                                    in_=att[:, :])

                        if f == 1:
                            for cc in range(KC):
                                nc.scalar.dma_start(
                                    out=wosel_sb[:, cc, :, :],
                                    in_=wosel[cc, :, :, :])

                    nc.gpsimd.collective_compute(
                        "AllToAll", mybir.AluOpType.bypass,
                        ins=[a2a_in.ap().opt()], outs=[a2a_out.ap().opt()],
                        replica_groups=RG)

                # ============ Phase O: o-projection for my chunk ============
                with tc.tile_pool(name="pcs", bufs=1) as pcs, \
                     tc.tile_pool(name="wop", bufs=2) as wop, \
                     tc.tile_pool(name="ops", bufs=4, space="PSUM") as opsp:
                    warm = opsp.tile([128, 512], F32, tag="warm")
                    for _ in range(100):
                        nc.tensor.matmul(warm[:, :], ones128[:, :],
                                         cosN_s[:, 0:512], start=True,
                                         stop=True)
                    pieces = pcs.tile([128, 16, QC], BF16, tag="pieces")
                    for src in range(NC):
                        for hh in range(2):
                            nc.sync.dma_start(
                                out=pieces[:, 2 * src + hh, :],
                                in_=a2a_out.ap()[src, hh, :, :])
                    for cc in range(KC):
                        ops = opsp.tile([128, QC], F32, tag="ops", name="ops")
                        for p in range(16):
                            nc.tensor.matmul(ops[:, :], wosel_sb[:, cc, p, :],
                                             pieces[:, p, :],
                                             start=(p == 0), stop=(p == 15))
                        osb = wop.tile([128, QC], F32, tag="osb", name="osb")
                        nc.scalar.copy(osb[:, :], ops[:, :])
                        nc.sync.dma_start(
                            out=outT.ap()[cc * 128:(cc + 1) * 128, :],
                            in_=osb[:, :])

    nc.compile()
    return nc


_NC_CACHE = {}


def _get_nc():
    if "nc" not in _NC_CACHE:
        _NC_CACHE["nc"] = build_kernel()
    return _NC_CACHE["nc"]


_PERM = np.concatenate([np.arange(0, 128, 2), np.arange(1, 128, 2)])


def _prep_inputs(x, freqs_cos, freqs_sin, Wq, bq, Wk, bk, Wv, bv, Wo, bo,
                 gq, gk, frame_seqlen):
    assert int(frame_seqlen) == L
    bf16 = ml_dtypes.bfloat16
    f16 = np.float16
    x2d = np.asarray(x, np.float32).reshape(T, D)
    Wq = np.asarray(Wq, np.float32)
    Wk = np.asarray(Wk, np.float32)
    Wv = np.asarray(Wv, np.float32)
    Wo = np.asarray(Wo, np.float32)
    bq = np.asarray(bq, np.float32)
    bk = np.asarray(bk, np.float32)
    bv = np.asarray(bv, np.float32)
    gq = np.asarray(gq, np.float32)
    gk = np.asarray(gk, np.float32)
    cos = np.asarray(freqs_cos, np.float32)  # [T, 64]
    sin = np.asarray(freqs_sin, np.float32)

    xT_nat = np.ascontiguousarray(
        x2d.T.reshape(KC, 128, T).transpose(1, 0, 2)).astype(bf16)

    # rope tables, natural order
    cosN_h = np.empty((128, T), np.float32)
    sinN_h = np.empty((128, T), np.float32)
    cosN_h[0:64] = cos.T
    cosN_h[64:128] = cos.T
    sinN_h[0:64] = -sin.T
    sinN_h[64:128] = sin.T

    maskb_h = np.zeros((128, NF), np.float32)
    for f in range(NF):
        maskb_h[TAILV[f]:, f] = -30000.0

    def wcols_perm(W, g, h):
        cols = 128 * h + _PERM
        Wp = W[:, cols] * g[cols][None, :]
        return Wp.reshape(KC, 128, 128)  # [dd, p, j]

    def wcols(W, h):
        return W[:, 128 * h:128 * (h + 1)].reshape(KC, 128, 128)

    in_maps = []
    for c in range(NC):
        hf, hs, hc = FULLH[c], SHARH[c], c % 2
        slot_idx = np.concatenate(
            [np.arange(L * f + QC * hc, L * f + QC * hc + QC)
             for f in range(NF)])
        xh2_h = np.ascontiguousarray(
            x2d[slot_idx].T.reshape(KC, 128, NF * QC)
            .transpose(1, 0, 2)).astype(bf16)

        def pack2(a, b):  # [KC,128,128] x2 -> [128, 2, KC, 128]
            return np.ascontiguousarray(
                np.stack([a, b], 0).transpose(2, 0, 1, 3)).astype(bf16)

        wq_h = pack2(wcols_perm(Wq, gq, hf), wcols_perm(Wq, gq, hs))
        wk_h = pack2(wcols_perm(Wk, gk, hf), wcols_perm(Wk, gk, hs))
        wv_h = pack2(wcols(Wv, hf), wcols(Wv, hs))

        bias_h = np.stack([
            bq[128 * hf + _PERM], bq[128 * hs + _PERM],
            bk[128 * hf + _PERM], bk[128 * hs + _PERM],
            bv[128 * hf:128 * (hf + 1)], bv[128 * hs:128 * (hs + 1)],
        ], 1).astype(np.float32)  # [128, 6]

        mqw_h = np.zeros((128, 2, 128), np.float32)
        mqw_h[:, hc, :] = 1.0

        maskq_h = np.zeros((128, NCH), np.float32)
        maskq_h[:, hc::2] = 1.0

        cosS_h = cosN_h[:, slot_idx]
        sinS_h = sinN_h[:, slot_idx]

        # wosel[cc, p, piece, j]; piece = 2*src + hh
        wosel_h = np.zeros((KC, 128, 16, 128), np.float32)
        for src in range(NC):
            rows_f = Wo[128 * FULLH[src]:128 * (FULLH[src] + 1), :]
            wosel_h[:, :, 2 * src + 0, :] = (
                rows_f.reshape(128, KC, 128).transpose(1, 0, 2))
            if (src % 2) == (c % 2):
                rows_s = Wo[128 * SHARH[src]:128 * (SHARH[src] + 1), :]
                wosel_h[:, :, 2 * src + 1, :] = (
                    rows_s.reshape(128, KC, 128).transpose(1, 0, 2))

        in_maps.append({
            "xT": xT_nat,
            "xh2": xh2_h,
            "wq": wq_h, "wk": wk_h, "wv": wv_h,
            "wosel": np.ascontiguousarray(wosel_h).astype(bf16),
            "biasqkv": bias_h,
            "mqw": mqw_h.astype(bf16),
            "cosN": cosN_h.astype(f16), "sinN": sinN_h.astype(f16),
            "cosS": np.ascontiguousarray(cosS_h).astype(f16),
            "sinS": np.ascontiguousarray(sinS_h).astype(f16),
            "maskb": maskb_h,
            "maskq": maskq_h,
        })
    return in_maps


def kernel(x, freqs_cos, freqs_sin, Wq, bq, Wk, bk, Wv, bv, Wo, bo,
           gq, gk, frame_seqlen):
    in_maps = _prep_inputs(x, freqs_cos, freqs_sin, Wq, bq, Wk, bk, Wv, bv,
                           Wo, bo, gq, gk, frame_seqlen)
    nc = _get_nc()
    res = run_bass_kernel_spmd(nc, in_maps, core_ids=list(range(NC)))
    bo = np.asarray(bo, np.float32)
    out = np.empty((1, T, D), np.float32)
    for c in range(NC):
        out[0, c * QC:(c + 1) * QC, :] = res.results[c]["outT"].T + bo[None, :]
    return out
